# revision 49
# baseline (speedup 1.0000x reference)
"""Trainium2 Bass kernel for nn_AttentionBlock (GroupNorm -> QKV -> cross+self
attention -> back projection + residual).

Sharding: data-parallel over batch B=8, one batch element per NeuronCore.

The hard floor is the Scalar (ACT) engine: 8 heads x 16 s-chunks of exp over
[128,1024] ~= 147us. Everything else is arranged to hide under that stream:
  * x cast to bf16 on host; DMAs spread over the 3 issue-capable rings
    (sync/gpsimd/scalar) in dependency-priority order; per-channel smalls are
    packed into one [128,32] tensor so they cost one descriptor.
  * GroupNorm stats run on DVE via accum_out as x chunks land; rstd =
    exp(-0.5*ln(var+eps)) on ACT so the kernel uses one activation table set.
  * Heads are processed in pairs: scores row-packed (two K=64 matmuls
    concurrent in row groups 0/64), PV col-packed (head A -> psum partitions
    0-63, head B -> 64-127 of one [128,1024] accumulator), and the softmax
    denominators via four concurrent M=1 col-tiled matmuls per chunk whose
    partials are folded into an SBUF accumulator by one DVE add.
  * One global slot stream: scores+exp for slot g, PV+Z trailing by TRAIL
    slots (crossing pair boundaries), phase-1 leftovers (v projections,
    deferred q/k/kc tiles) spread across early slots. PSUM: 3-buffer ring of
    [128,1024] tiles (6 banks, shared by scores/vt/proj/z) + PV pair
    accumulator (2 banks).
  * 1/Z via reciprocal_approx_fast + gpsimd partition_broadcast; back
    projection + residual at the end on all 8 banks.
"""

import contextlib
import functools

import numpy as np
import ml_dtypes

import concourse.bacc as bacc
import concourse.bass as bass
import concourse.tile as tile
from concourse import mybir
from concourse import bass_utils

BF16 = ml_dtypes.bfloat16
F32 = mybir.dt.float32
BF = mybir.dt.bfloat16
AF = mybir.ActivationFunctionType
ALU = mybir.AluOpType
AX = mybir.AxisListType

C = 512
T = 1024
S = 1024
NH = 8
HS = 64
EPS = 1e-5
NK = 4          # 128-partition channel chunks
NSC = 16        # 128-row score s-chunks (self 0..7, cond 8..15)
GSIZE = 16      # channels per group
CHUNK_ORDER = list(range(8)) + list(range(8, 16))   # self chunks first
TRAIL = 6       # PV+Z trail the score/exp stream by this many slots


def _build_body(nc, tc, d, sbuf):
    pers = sbuf.enter_context(tc.tile_pool(name="pers", bufs=1))
    work = sbuf.enter_context(tc.tile_pool(name="work", bufs=2))
    epool = sbuf.enter_context(tc.tile_pool(name="epool", bufs=16))
    rzpool = sbuf.enter_context(tc.tile_pool(name="rzpool", bufs=2))
    outp = sbuf.enter_context(tc.tile_pool(name="outp", bufs=2))

    # ---------------- DMA loads ----------------
    # Only sync/gpsimd/scalar can issue DMAs; each ring sustains ~66 GB/s and
    # processes its queue in order, so order = priority.
    x_sb = [pers.tile([128, T], BF, tag=f"x{j}", name=f"x_sb{j}")
            for j in range(NK)]

    def load_w(key, eng, kks=range(NK), tiles=None):
        tiles = tiles if tiles is not None else [None] * NK
        for kk in kks:
            t_ = pers.tile([128, 512], BF, tag=f"{key}{kk}", name=f"{key}_sb{kk}")
            eng.dma_start(t_[:], d[key][128 * kk:128 * (kk + 1), :])
            tiles[kk] = t_
        return tiles

    #  sync:   x0, wq0, x3, wq1, bvh, bvch, wvc, wb
    #  gpsimd: x1, wq2, wq3, wk2, wk3, wkc
    #  scalar: x2, smallpack, selb, wk0, wk1, wv, cond
    nc.sync.dma_start(x_sb[0][:], d["x"][0:128, :])
    nc.gpsimd.dma_start(x_sb[1][:], d["x"][128:256, :])
    nc.scalar.dma_start(x_sb[2][:], d["x"][256:384, :])
    wq_sb = load_w("wq", nc.sync, kks=(0,))
    smallpack = pers.tile([128, 32], F32, tag="smallpack", name="smallpack")
    nc.scalar.dma_start(smallpack[:], d["smallpack"][:])
    sel_b = pers.tile([8, 128], F32, tag="selb", name="selb_sb")
    nc.scalar.dma_start(sel_b[:], d["sel_b"][:])
    nc.sync.dma_start(x_sb[3][:], d["x"][384:512, :])
    load_w("wq", nc.gpsimd, kks=(2, 3), tiles=wq_sb)
    wk_sb = load_w("wk", nc.scalar, kks=(0, 1))
    load_w("wq", nc.sync, kks=(1,), tiles=wq_sb)
    load_w("wk", nc.gpsimd, kks=(2, 3), tiles=wk_sb)
    bvh_sb = pers.tile([1, 512], BF, tag="bvh", name="bvh_sb")
    nc.sync.dma_start(bvh_sb[:], d["bvh"][:])
    bvch_sb = pers.tile([1, 512], BF, tag="bvch", name="bvch_sb")
    nc.sync.dma_start(bvch_sb[:], d["bvch"][:])
    wv_sb = load_w("wv", nc.scalar)
    wkc_sb = load_w("wkc", nc.gpsimd)
    wvc_sb = load_w("wvc", nc.sync)
    cond_sb = []
    for j in range(NK):
        t_ = pers.tile([128, S], BF, tag=f"cond{j}", name=f"cond_sb{j}")
        nc.scalar.dma_start(t_[:], d["cond"][128 * j:128 * (j + 1), :])
        cond_sb.append(t_)
    wb_sb = load_w("wb", nc.sync)

    gamma_sb = smallpack[:, 0:4]
    beta_sb = smallpack[:, 4:8]
    bq_sb = smallpack[:, 8:12]
    bk_sb = smallpack[:, 12:16]
    bkc_sb = smallpack[:, 16:20]
    bb_sb = smallpack[:, 20:24]
    sel_f = smallpack[:, 24:32]

    # v-bias rows broadcast to 128 partitions on gpsimd
    bvb = pers.tile([128, 512], BF, tag="bvb", name="bvb")
    nc.gpsimd.partition_broadcast(bvb[:], bvh_sb[0:1, :])
    bvcb = pers.tile([128, 512], BF, tag="bvcb", name="bvcb")
    nc.gpsimd.partition_broadcast(bvcb[:], bvch_sb[0:1, :])

    # ---------------- persistent SBUF tiles ----------------
    stats = pers.tile([128, 8], F32, tag="stats", name="stats")
    epsc = pers.tile([128, 1], F32, tag="epsc", name="epsc")
    nc.vector.memset(epsc[:], EPS)
    xn_sb = [pers.tile([128, T], BF, tag=f"xn{j}", name=f"xn_sb{j}")
             for j in range(NK)]
    kc_sb = [pers.tile([128, T], BF, tag=f"kc{m}", name=f"kc_sb{m}")
             for m in range(4)]
    q_sb = [pers.tile([128, T], BF, tag=f"q{m}", name=f"q_sb{m}")
            for m in range(4)]
    k_sb = [pers.tile([128, T], BF, tag=f"k{m}", name=f"k_sb{m}")
            for m in range(4)]
    vt_sb = [pers.tile([128, 8, 64], BF, tag=f"vt{i}", name=f"vt_sb{i}")
             for i in range(NSC)]
    ones_sb = pers.tile([128, 1], BF, tag="ones", name="ones_sb")
    nc.gpsimd.memset(ones_sb[:], 1.0)
    attn_sb = [pers.tile([128, T], BF, tag=f"attn{p}", name=f"attn_sb{p}")
               for p in range(4)]

    # ---------------- psum: ring (6 banks) + pv pair accumulator (2) --------
    ps_stack = sbuf.enter_context(contextlib.ExitStack())
    ring = ps_stack.enter_context(
        tc.tile_pool(name="ring", bufs=3, space="PSUM"))
    ps_pv = ps_stack.enter_context(
        tc.tile_pool(name="ps_pv", bufs=1, space="PSUM"))
    rn = [0]

    def rtile():
        rn[0] += 1
        return ring.tile([128, T], F32, tag="ring", name=f"rg{rn[0]}")

    # PE warm-up: junk matmuls on x0 keep the HAM activity window busy while
    # the GroupNorm chain resolves, so the critical q/k projections run at
    # the 2.4 GHz clock. Their output is never read.
    jtile = ring.tile([128, T], F32, tag="ring", name="junk")
    for _ in range(20):
        nc.tensor.matmul(jtile[:, 0:512], x_sb[0][:, 0:128], x_sb[0][:, 0:512],
                         start=True, stop=True)

    # ---------------- GroupNorm stats (as x chunks land) ----------------
    for j in range(NK):
        sq = work.tile([128, T], BF, tag="sq", name=f"sq{j}")
        nc.vector.scalar_tensor_tensor(
            sq[:], x_sb[j][:], 1.0, x_sb[j][:], op0=ALU.mult, op1=ALU.mult,
            accum_out=stats[:, 4 + j:5 + j])
        xc = work.tile([128, T], BF, tag="xc", name=f"xc{j}")
        nc.vector.tensor_scalar(xc[:], x_sb[j][:], 1.0, 0.0, op0=ALU.mult,
                                op1=ALU.add, accum_out=stats[:, j:j + 1])

    # preload the ln/exp activation table while DMAs stream
    preld = pers.tile([1, 1], F32, tag="preld", name="preld")
    nc.scalar.activation(preld[:], epsc[0:1, :], AF.Ln)

    # ---------------- projections / vt through the psum ring ----------------
    def proj_half(w_tiles, rhs_tiles, bias_sb, out_sb, m, t2):
        ps = rtile()
        for kk in range(NK):
            nc.tensor.matmul(
                ps[:, 0:512], w_tiles[kk][:, 128 * m:128 * (m + 1)],
                rhs_tiles[kk][:, 512 * t2:512 * (t2 + 1)],
                start=(kk == 0), stop=(kk == NK - 1))
        nc.vector.tensor_scalar(
            out_sb[:, 512 * t2:512 * (t2 + 1)], ps[:, 0:512],
            bias_sb[:, m:m + 1], None, op0=ALU.add)

    def proj_m(w_tiles, rhs_tiles, bias_sb, out_sb, m):
        for t2 in range(2):
            proj_half(w_tiles, rhs_tiles, bias_sb, out_sb, m, t2)

    def emit_vt(i):
        m8 = i % 8
        src = xn_sb if i < 8 else cond_sb
        w = wv_sb if i < 8 else wvc_sb
        bcast = bvb if i < 8 else bvcb
        ps = rtile()
        for kk in range(NK):
            nc.tensor.matmul(ps[:, 0:512], src[kk][:, 128 * m8:128 * (m8 + 1)],
                             w[kk][:], start=(kk == 0), stop=(kk == NK - 1))
        nc.vector.tensor_add(
            vt_sb[i][:, :, 0:64],
            ps[:, 0:512].rearrange("p (h c) -> p h c", h=NH),
            bcast[:].rearrange("p (h c) -> p h c", h=NH))

    # ---------------- attention machinery ----------------
    def emit_scores(p, i):
        """Row-packed scores for head pair (2p, 2p+1), s-chunk i: the two K=64
        matmuls run concurrently in row groups 0-63 / 64-127."""
        ksrc = k_sb[p] if i < 8 else kc_sb[p]
        scol = 128 * (i % 8)
        scA = rtile()
        scB = rtile()
        for t2 in range(2):
            for rb, sc in ((0, scA), (64, scB)):
                nc.tensor.matmul(
                    sc[:, 512 * t2:512 * (t2 + 1)],
                    ksrc[rb:rb + 64, scol:scol + 128],
                    q_sb[p][rb:rb + 64, 512 * t2:512 * (t2 + 1)],
                    start=True, stop=True)
        eA = epool.tile([128, T], BF, tag="e", name=f"eA{p}_{i}")
        nc.scalar.activation(eA[:], scA[:], AF.Exp, scale=0.125)
        eB = epool.tile([128, T], BF, tag="e", name=f"eB{p}_{i}")
        nc.scalar.activation(eB[:], scB[:], AF.Exp, scale=0.125)
        return eA, eB

    def emit_pvz(p, pvp, zacc, epair, i, ci):
        """Col-packed PV (head A -> partitions 0-63, head B -> 64-127) plus 4
        concurrent M=1 col-tiled Z-partial matmuls folded into zacc on DVE."""
        eA, eB = epair
        st, sp = (ci == 0), (ci == NSC - 1)
        for t2 in range(2):
            cs = slice(512 * t2, 512 * (t2 + 1))
            nc.tensor.matmul(pvp[0:64, cs], vt_sb[i][:, 2 * p, 0:64],
                             eA[:, cs], start=st, stop=sp)
            nc.tensor.matmul(pvp[64:128, cs], vt_sb[i][:, 2 * p + 1, 0:64],
                             eB[:, cs], start=st, stop=sp)
        if ci % 2 == 0:
            zc = zcur[0] = rtile()
        else:
            zc = zcur[0]
        for row, e, t2 in ((0, eA, 0), (32, eA, 1), (64, eB, 0), (96, eB, 1)):
            nc.tensor.matmul(zc[row:row + 1, 0:512], ones_sb[:],
                             e[:, 512 * t2:512 * (t2 + 1)],
                             start=(ci % 2 == 0), stop=(ci % 2 == 1),
                             tile_position=(0, row))
        # only rows 0/32/64/96 of zc are meaningful; the rest accumulates
        # stale psum data that is never read
        if ci == 1:
            nc.vector.tensor_copy(zacc[:], zc[:, 0:512])
        elif ci % 2 == 1:
            nc.vector.tensor_add(zacc[:], zacc[:], zc[:, 0:512])

    def pair_drain(p, pvp, zacc):
        """Copy unnormalized PV out first (frees the psum banks fast), then
        normalize both heads in SBUF."""
        nc.vector.tensor_copy(attn_sb[p][:], pvp[0:128, 0:T])
        zrow = rzpool.tile([1, 2 * T], F32, tag="zrow", name=f"zrow{p}")
        for n, row in enumerate((0, 32, 64, 96)):
            nc.vector.tensor_copy(zrow[0:1, 512 * n:512 * (n + 1)],
                                  zacc[row:row + 1, :])
        zb = rzpool.tile([64, T], F32, tag="zb", name=f"zb{p}")
        rz = rzpool.tile([64, T], F32, tag="rz", name=f"rz{p}")
        nc.gpsimd.partition_broadcast(zb[:], zrow[0:1, 0:T])
        nc.vector.reciprocal_approx_fast(rz[:], zb[:])
        nc.vector.tensor_mul(attn_sb[p][0:64, :], attn_sb[p][0:64, :], rz[:])
        zbB = rzpool.tile([64, T], F32, tag="zbB", name=f"zbB{p}")
        rzB = rzpool.tile([128, T], F32, tag="rzB", name=f"rzB{p}")
        nc.gpsimd.partition_broadcast(zbB[:], zrow[0:1, T:2 * T])
        nc.vector.reciprocal_approx_fast(rzB[0:64, :], zbB[:])
        nc.vector.tensor_copy(rzB[64:128, :], rzB[0:64, :])
        nc.vector.tensor_mul(attn_sb[p][64:128, :], attn_sb[p][64:128, :],
                             rzB[64:128, :])

    # ---------------- GroupNorm combine + xn ----------------
    gpst = rtile()
    nc.tensor.matmul(gpst[0:8, 0:8], sel_f, stats[:], start=True, stop=True)
    gstats = pers.tile([8, 8], F32, tag="gstats", name="gstats")
    inv_n = 1.0 / (GSIZE * T)
    nc.vector.tensor_scalar_mul(gstats[:, 0:8], gpst[0:8, 0:8], inv_n)
    var = pers.tile([8, 4], F32, tag="var", name="var")
    nc.vector.tensor_mul(var[:], gstats[:, 0:4], gstats[:, 0:4])
    nc.vector.tensor_sub(var[:], gstats[:, 4:8], var[:])
    # rstd = exp(-0.5 * ln(var + eps)): stays in the exp/ln table set
    nc.scalar.activation(var[:], var[:], AF.Ln, bias=epsc[0:8, :])
    nc.scalar.activation(gstats[:, 4:8], var[:], AF.Exp, scale=-0.5)
    bpst = rtile()
    nc.tensor.matmul(bpst[0:128, 0:8], sel_b[:], gstats[:], start=True,
                     stop=True)
    for _ in range(12):
        nc.tensor.matmul(jtile[:, 0:512], x_sb[0][:, 0:128], x_sb[0][:, 0:512],
                         start=True, stop=True)
    scale = pers.tile([128, 4], F32, tag="scale", name="scale")
    shift = pers.tile([128, 4], F32, tag="shift", name="shift")
    nc.vector.tensor_mul(scale[:], gamma_sb, bpst[0:128, 4:8])
    nc.vector.tensor_mul(shift[:], bpst[0:128, 0:4], scale[:])
    nc.vector.tensor_sub(shift[:], beta_sb, shift[:])
    for j in range(NK):
        nc.vector.tensor_scalar(xn_sb[j][:], x_sb[j][:], scale[:, j:j + 1],
                                shift[:, j:j + 1], op0=ALU.mult, op1=ALU.add)

    proj_m(wq_sb, xn_sb, bq_sb, q_sb[0], 0)
    proj_m(wk_sb, xn_sb, bk_sb, k_sb[0], 0)

    # ---------------- pair loop (one global slot stream) ----------------
    squeue = []       # (p, epair, i, ci) awaiting PV+Z
    pend = []         # deferred half-projection emitters
    pvzt = {}
    zcur = [None]
    for g in range(64 + TRAIL):
        if g < 64:
            p, ci = divmod(g, 16)
            i = CHUNK_ORDER[ci]
            if p == 0:
                emit_vt(i)          # self vts on slots 0-7, cond on 8-15
                if ci == 5:
                    proj_half(wkc_sb, cond_sb, bkc_sb, kc_sb[0], 0, 0)
                if ci == 6:
                    proj_half(wkc_sb, cond_sb, bkc_sb, kc_sb[0], 0, 1)
            if ci == 0 and p < 3:
                mn = p + 1
                pend = [(w, r, b, o, mn, t2)
                        for (w, r, b, o) in ((wkc_sb, cond_sb, bkc_sb, kc_sb[mn]),
                                             (wq_sb, xn_sb, bq_sb, q_sb[mn]),
                                             (wk_sb, xn_sb, bk_sb, k_sb[mn]))
                        for t2 in range(2)]
            if pend and ci in (2, 4, 7, 9, 11, 13):
                proj_half(*pend.pop(0))
            squeue.append((p, emit_scores(p, i), i, ci))
        if g >= TRAIL:
            pp, ep, ii, cci = squeue[g - TRAIL]
            if cci == 0:
                pvzt[pp] = (ps_pv.tile([128, T], F32, tag="pv", name=f"pv{pp}"),
                            rzpool.tile([128, 512], F32, tag="zacc",
                                        name=f"zacc{pp}"))
            emit_pvz(pp, pvzt[pp][0], pvzt[pp][1], ep, ii, cci)
            if cci == NSC - 1:
                pair_drain(pp, pvzt[pp][0], pvzt[pp][1])

    # ---------------- back projection + residual ----------------
    # Through the same psum ring (no pool-close barrier): pairs 0-2 partials
    # run under the tail of the exp stream; only the pair-3 matmuls serialize
    # behind the last drain.
    def bp_kk(bk, m, kk):
        for t2 in range(2):
            nc.tensor.matmul(bk[:, 512 * t2:512 * (t2 + 1)],
                             wb_sb[kk][:, 128 * m:128 * (m + 1)],
                             attn_sb[kk][:, 512 * t2:512 * (t2 + 1)],
                             start=(kk == 0), stop=(kk == NK - 1))

    def bp_out(bk, m):
        outsb = outp.tile([128, T], F32, tag="outsb", name=f"outsb{m}")
        for t2 in range(2):
            nc.vector.scalar_tensor_tensor(
                outsb[:, 512 * t2:512 * (t2 + 1)],
                bk[:, 512 * t2:512 * (t2 + 1)], bb_sb[:, m:m + 1],
                x_sb[m][:, 512 * t2:512 * (t2 + 1)],
                op0=ALU.add, op1=ALU.add)
            eng = nc.sync if t2 == 0 else nc.gpsimd
            eng.dma_start(
                d["out"][128 * m:128 * (m + 1), 512 * t2:512 * (t2 + 1)],
                outsb[:, 512 * t2:512 * (t2 + 1)])

    bks = {}
    for m in range(3):
        bks[m] = rtile()
        for kk in range(3):
            bp_kk(bks[m], m, kk)
    for m in range(3):
        bp_kk(bks[m], m, 3)
        bp_out(bks[m], m)
    bks[3] = rtile()
    for kk in range(4):
        bp_kk(bks[3], 3, kk)
    bp_out(bks[3], 3)


@functools.lru_cache(maxsize=1)
def _build():
    nc = bacc.Bacc("TRN2", target_bir_lowering=False, debug=False)
    d = {}
    d["x"] = nc.dram_tensor("x", [C, T], BF, kind="ExternalInput")
    d["cond"] = nc.dram_tensor("cond", [512, S], BF, kind="ExternalInput")
    for w in ("wq", "wk", "wkc", "wv", "wvc", "wb"):
        d[w] = nc.dram_tensor(w, [512, 512], BF, kind="ExternalInput")
    d["smallpack"] = nc.dram_tensor("smallpack", [128, 32], F32,
                                    kind="ExternalInput")
    d["sel_b"] = nc.dram_tensor("sel_b", [8, 128], F32, kind="ExternalInput")
    d["bvh"] = nc.dram_tensor("bvh", [1, 512], BF, kind="ExternalInput")
    d["bvch"] = nc.dram_tensor("bvch", [1, 512], BF, kind="ExternalInput")
    d["out"] = nc.dram_tensor("out", [C, T], F32, kind="ExternalOutput")

    with tile.TileContext(nc) as tc:
        with contextlib.ExitStack() as sbuf:
            _build_body(nc, tc, d, sbuf)
    nc.compile()
    return nc


def _prep_shared(gn_gamma, gn_beta, Wf, bf, Wt, bt, Wb, bb):
    f32 = np.float32
    Wf_r = np.asarray(Wf, f32).reshape(8, 3, 64, 512)
    Wt_r = np.asarray(Wt, f32).reshape(8, 2, 64, 512)
    bf_r = np.asarray(bf, f32).reshape(8, 3, 64)
    bt_r = np.asarray(bt, f32).reshape(8, 2, 64)

    def wT(a):  # [512(out), 512(in)] -> [in, out] bf16
        return np.ascontiguousarray(a.reshape(512, 512).T).astype(BF16)

    def pcol(v):  # [512] -> [128, 4]
        return np.ascontiguousarray(np.asarray(v, f32).reshape(4, 128).T)

    sel_f = (np.arange(128)[:, None] // GSIZE ==
             np.arange(8)[None, :]).astype(f32)
    smallpack = np.concatenate([
        pcol(gn_gamma), pcol(gn_beta),
        pcol(bf_r[:, 0].reshape(512)), pcol(bf_r[:, 1].reshape(512)),
        pcol(bt_r[:, 0].reshape(512)), pcol(bb), sel_f], axis=1)
    return {
        "wq": wT(Wf_r[:, 0]),
        "wk": wT(Wf_r[:, 1]),
        "wv": wT(Wf_r[:, 2]),
        "wkc": wT(Wt_r[:, 0]),
        "wvc": wT(Wt_r[:, 1]),
        "wb": np.ascontiguousarray(np.asarray(Wb, f32).T).astype(BF16),
        "smallpack": np.ascontiguousarray(smallpack),
        "sel_b": np.ascontiguousarray(sel_f.T),
        "bvh": np.ascontiguousarray(bf_r[:, 2].reshape(1, 512)).astype(BF16),
        "bvch": np.ascontiguousarray(bt_r[:, 1].reshape(1, 512)).astype(BF16),
    }


def _run(inputs, trace=False, tmpdir=None):
    nc = _build()
    shared = _prep_shared(inputs["gn_gamma"], inputs["gn_beta"],
                          inputs["Wf"], inputs["bf"], inputs["Wt"],
                          inputs["bt"], inputs["Wb"], inputs["bb"])
    feat = np.asarray(inputs["input_feature"], np.float32)
    cond = np.asarray(inputs["attention_condition"], np.float32)
    in_maps = []
    for b in range(8):
        m = dict(shared)
        m["x"] = feat[b].reshape(C, T).astype(BF16)
        m["cond"] = cond[b].astype(BF16)
        in_maps.append(m)
    res = bass_utils.run_bass_kernel_spmd(nc, in_maps, core_ids=list(range(8)),
                                          trace=trace, tmpdir=tmpdir)
    out = np.stack([r["out"] for r in res.results], axis=0)
    return out.reshape(8, C, 32, 32).astype(np.float32), res


def kernel(**inputs):
    out, _ = _run(inputs, trace=False)
    return out
